# revision 2
# baseline (speedup 1.0000x reference)
"""Multi-head attention (B=4, S=2048, D=1024, H=16) on 8 NeuronCores — v3.2.

Sharding: core c = (batch b = c//2, head-half hh = c%2).  Each core computes
Q/K/V for its 8 heads over the FULL sequence, then a PARTIAL output
projection; the two cores of a batch pair are summed on the host (the
tensor-parallel all-reduce).

Pipeline: 256 steps, one per (head, qblock, query-half n, chunk-pair u).
Each step's two score matmuls — even key chunk 2u (KT partitions 0:64, PE
row group 0) and odd chunk 2u+1 (partitions 64:128, row group 1) — write
one [128,1024] PSUM tile freed by ONE exp, so they become ready together,
sit adjacent in the PE queue, and run concurrently in disjoint 64-row PE
groups (row tiling ~2x).  QT is duplicated into both partition halves to
match the stationary's row group.  attn@V units are FIFO-paced (elastic
lag) and issued in pairs so same-stationary-class MMs batch (fewer exposed
LDWEIGHTS).  All projections run as ~4-8 MM quanta spread across steps so
the exp stream (~1us per [128,1024], the pace-setter) never stalls.

Inputs arrive as ONE large strided DMA per tensor (row-blocks of the DRAM
tensor gathered into free-dim slots) — 10 issues instead of 50, avoiding
the shared-semaphore-pool serialization that cost ~35us of prologue.

QK_FP8: Q/K projections contract x in fp8e4 DoubleRow (256 dims/MM).
FP8_HEADS: heads below this index run attn@V in fp8 DoubleRow (one MM per
chunk-pair, 256 keys) with exp writing fp8 probs directly.  Error budget
(measured, rel): bf16-only 0.0023; +QK fp8 0.0164; +attnv fp8 0.0187;
both 0.0248 (fails 2e-2) — enable at most one.

Softmax: row sums via a ones-block in the attn@V stationary (M<=128 adds
no PE cycles); normalization = DVE reciprocal_approx_fast (must run at
partition base 0 — custom-ucode quirk) + tensor_mul.  V-bias is folded
into the out bias host-side (attention rows sum to 1).
"""

import numpy as np
import ml_dtypes
from contextlib import ExitStack

P = 128
DM = 1024
SEQ = 2048
H = 16
H8 = 8          # heads per core
NPAIR = 4      # head pairs per core
DK = 64
NCORES = 8
NQB = 2         # query blocks of 1024
NU = 8          # chunk-pair steps per (head, qblock, n-half)
NSTEP = H8 * NQB * 2 * NU   # 256
LAG = 3         # min attnv trail (steps)

FP8_HEADS = 8   # per-core heads (from index 0) using fp8 DoubleRow attn@V
QK_FP8 = False   # Q/K projections contract x in fp8 DoubleRow
DBG = False     # add DRAM taps of intermediate tiles

_BF16 = ml_dtypes.bfloat16
_FP8 = ml_dtypes.float8_e4m3

_CACHE = {}


def _decode(t):
    """step/unit index -> (bi, qb, j, par, h, n, u)"""
    bi, w = divmod(t, 2 * NU)
    qb, jj = divmod(bi, H8)
    j, par = divmod(jj, 2)
    n, u = divmod(w, NU)
    return bi, qb, j, par, 2 * j + par, n, u


def _build():
    from concourse import bacc
    import concourse.mybir as mybir
    from concourse.tile import TileContext

    dt = mybir.dt
    f32 = dt.float32
    bf16 = dt.bfloat16
    fp8 = dt.float8e4
    AF = mybir.ActivationFunctionType
    DR = mybir.MatmulPerfMode.DoubleRow

    nc = bacc.Bacc("TRN2", target_bir_lowering=False, debug=False)

    # all inputs arrive pre-baked in their SBUF layout ([128, nblk*cols],
    # row-blocks in the free dim) so each is ONE contiguous full-bw DMA
    xT_d = nc.dram_tensor("xT", [P, 8 * SEQ], bf16, kind="ExternalInput")
    wv_d = nc.dram_tensor("wvT", [P, 8 * 512], bf16, kind="ExternalInput")
    wo_d = nc.dram_tensor("woT", [P, 4 * DM], bf16, kind="ExternalInput")
    bq_d = nc.dram_tensor("bq4", [P, NPAIR], f32, kind="ExternalInput")
    bk_d = nc.dram_tensor("bk4", [P, NPAIR], f32, kind="ExternalInput")
    bo_d = nc.dram_tensor("bob", [P, DM], f32, kind="ExternalInput")
    if QK_FP8:
        x8_d = nc.dram_tensor("xT8", [P, 4 * 2 * SEQ], fp8, kind="ExternalInput")
        wq_d = nc.dram_tensor("wq8", [P, 4 * 2 * 512], fp8, kind="ExternalInput")
        wk_d = nc.dram_tensor("wk8", [P, 4 * 2 * 512], fp8, kind="ExternalInput")
    else:
        wq_d = nc.dram_tensor("wqT", [P, 8 * 512], bf16, kind="ExternalInput")
        wk_d = nc.dram_tensor("wkT", [P, 8 * 512], bf16, kind="ExternalInput")
    out_d = nc.dram_tensor("out", [SEQ, DM], f32, kind="ExternalOutput")
    if DBG:
        dkt_d = nc.dram_tensor("dkt", [P, 1024 * H8], bf16, kind="ExternalOutput")
        dqt_d = nc.dram_tensor("dqt", [P, 1024 * H8 * NQB], bf16, kind="ExternalOutput")
        dvb_d = nc.dram_tensor("dvb", [P, 768 * 16], bf16, kind="ExternalOutput")
        dvt_d = nc.dram_tensor("dvt", [P, SEQ * NPAIR], bf16, kind="ExternalOutput")

    use_f8 = FP8_HEADS > 0
    use_b16 = FP8_HEADS < H8

    with TileContext(nc) as tc, ExitStack() as ctx:
        # ---- permanent pools ----
        kt_pool = ctx.enter_context(tc.tile_pool(name="kt", bufs=H8))
        qt_pool = ctx.enter_context(tc.tile_pool(name="qt", bufs=H8 * NQB))
        vt_pool = ctx.enter_context(tc.tile_pool(name="vt", bufs=NPAIR))
        vb_pool = ctx.enter_context(tc.tile_pool(name="vb", bufs=16 if use_b16 else 1))
        v8_pool = ctx.enter_context(tc.tile_pool(name="v8", bufs=8 if use_f8 else 1))
        pt_pool = ctx.enter_context(tc.tile_pool(name="pt", bufs=20 if use_b16 else 1))
        p8_pool = ctx.enter_context(tc.tile_pool(name="p8", bufs=20 if use_f8 else 1))
        rc_pool = ctx.enter_context(tc.tile_pool(name="rc", bufs=2))
        misc = ctx.enter_context(tc.tile_pool(name="mi", bufs=1))
        wqk_pool = ctx.enter_context(tc.tile_pool(name="w8", bufs=1))
        if QK_FP8:
            x8_pool = ctx.enter_context(tc.tile_pool(name="x8", bufs=1))
        # PSUM (8 banks): scores 2x[128,1024] (4) + proj 2x[128,512] (2)
        #                 + attnv accum 2x[128,512] (2)
        psc = ctx.enter_context(tc.tile_pool(name="psc", bufs=2, space="PSUM"))
        ppj = ctx.enter_context(tc.tile_pool(name="ppj", bufs=2, space="PSUM"))
        pvq = ctx.enter_context(tc.tile_pool(name="pvq", bufs=2, space="PSUM"))

        KT = [kt_pool.tile([P, 1024], bf16, tag="kt", name="kt") for _ in range(H8)]
        QT = [[qt_pool.tile([P, 1024], bf16, tag="qt", name="qt")
               for _ in range(NQB)] for _ in range(H8)]
        VT = [vt_pool.tile([P, SEQ], bf16, tag="vt", name="vt") for _ in range(NPAIR)]
        VB = ([vb_pool.tile([P, NPAIR * 192], bf16, tag="vb", name="vb")
               for _ in range(16)] if use_b16 else None)
        V8 = ([v8_pool.tile([P, 2 * NPAIR * 192], fp8, tag="v8", name="v8")
               for _ in range(8)] if use_f8 else None)

        # ---- consolidated input DMAs (one contiguous transfer per tensor).
        # Scalar queue carries ONLY activations; weights/x8 on sync.
        def load_blocked(pool, tag, dram, nblk, cols, dtp, eng, ndma=1):
            t = pool.tile([P, nblk * cols], dtp, tag=tag, name=tag)
            n = nblk * cols
            for i in range(ndma):
                sl = slice(i * n // ndma, (i + 1) * n // ndma)
                eng.dma_start(t[:, sl], dram[:, sl])
            return t, t[:].rearrange("p (k c) -> p k c", k=nblk)

        if QK_FP8:
            # x8 free-dim order (thalf, k2, jj, 1024): the prologue's K/Q
            # bursts (keys/queries 0:1024) unblock after the first 1MB dma
            _, WKv = load_blocked(wqk_pool, "wk8", wk_d, 4, 1024, fp8, nc.sync)
            x8t = x8_pool.tile([P, 4 * 2 * SEQ], fp8, tag="x8", name="x8")
            nc.sync.dma_start(x8t[:, 0:8192], x8_d[:, 0:8192])
            _, WQv = load_blocked(wqk_pool, "wq8", wq_d, 4, 1024, fp8, nc.sync)
            X8v = x8t[:].rearrange("p (hf k2 jj t) -> p hf k2 jj t", hf=2, k2=4, jj=2)
        else:
            _, WKv = load_blocked(wqk_pool, "wkT", wk_d, 8, 512, bf16, nc.sync)
            _, WQv = load_blocked(wqk_pool, "wqT", wq_d, 8, 512, bf16, nc.sync)
        bq_s = misc.tile([P, NPAIR], f32, tag="bq", name="bq")
        nc.sync.dma_start(bq_s[:], bq_d[:])
        bk_s = misc.tile([P, NPAIR], f32, tag="bk", name="bk")
        nc.sync.dma_start(bk_s[:], bk_d[:])
        if QK_FP8:
            nc.sync.dma_start(x8t[:, 8192:16384], x8_d[:, 8192:16384])

        if QK_FP8:
            def qk_mms(WV_, j, ps, psl, cols):
                hf, tt = divmod(cols.start, 1024)
                for k2 in range(4):
                    w = WV_[:, k2].rearrange("p (jj m) -> p jj m", jj=2)[:, :, j * P:(j + 1) * P]
                    x = X8v[:, hf, k2, :, tt:tt + 512]
                    nc.tensor.matmul(ps[:, psl], w, x, start=(k2 == 0),
                                     stop=(k2 == 3), perf_mode=DR)
        else:
            def qk_mms(WV_, j, ps, psl, cols):
                for k in range(8):
                    nc.tensor.matmul(
                        ps[:, psl], WV_[:, k, j * P:(j + 1) * P],
                        XT[k][:, cols], start=(k == 0), stop=(k == 7))

        def k_dve(ps, j, kh, n):
            """parity-split DVE writes of a K-proj [128, 512*w] psum region
            covering chunks from (kh*8 + n*4)."""
            pw = ps.rearrange("p (u par i) -> p u par i", par=2, i=P)
            c0 = (kh * 4 + n * 2) * P
            w = ps.shape[1] // 256
            for hp in range(2):
                h = 2 * j + hp
                r0 = hp * DK
                for par in range(2):
                    dst = KT[h][par * DK:(par + 1) * DK, c0:c0 + w * P]
                    nc.vector.tensor_scalar_add(
                        dst.rearrange("p (u i) -> p u i", i=P),
                        pw[r0:r0 + DK, :, par, :], bk_s[r0:r0 + DK, j:j + 1])

        def q_dve(ps, j, qb, n, wd):
            for hp in range(2):
                h = 2 * j + hp
                r0 = hp * DK
                for half in range(2):
                    nc.vector.tensor_scalar_add(
                        QT[h][qb][half * DK:(half + 1) * DK,
                                  n * 512:n * 512 + wd],
                        ps[r0:r0 + DK, :], bq_s[r0:r0 + DK, j:j + 1])

        def kproj_q(j, kh, n):
            ps = ppj.tile([P, 512], f32, tag="pj", name="pj")
            qk_mms(WKv, j, ps, slice(0, 512),
                   slice(kh * 1024 + n * 512, kh * 1024 + (n + 1) * 512))
            k_dve(ps[:], j, kh, n)

        def qproj_q(j, qb, n):
            ps = ppj.tile([P, 512], f32, tag="pj", name="pj")
            qk_mms(WQv, j, ps, slice(0, 512),
                   slice(qb * 1024 + n * 512, qb * 1024 + (n + 1) * 512))
            q_dve(ps[:], j, qb, n, 512)

        def kproj_wide(j, kh):
            """prologue K-proj half on the (idle) score psum pool."""
            ps = psc.tile([P, 1024], f32, tag="sc", name="sc")
            for n in range(2):
                qk_mms(WKv, j, ps, slice(n * 512, (n + 1) * 512),
                       slice(kh * 1024 + n * 512, kh * 1024 + (n + 1) * 512))
            k_dve(ps[:], j, kh, 0)

        def qproj_wide(j, qb):
            ps = psc.tile([P, 1024], f32, tag="sc", name="sc")
            for n in range(2):
                qk_mms(WQv, j, ps, slice(n * 512, (n + 1) * 512),
                       slice(qb * 1024 + n * 512, qb * 1024 + (n + 1) * 512))
            q_dve(ps[:], j, qb, 0, 1024)

        probs = {}

        def scores_step(s):
            bi, qb, j, par, h, n, u = _decode(s)
            f8 = h < FP8_HEADS
            ps = psc.tile([P, 1024], f32, tag="sc", name="sc")
            nc.tensor.matmul(
                ps[:, 0:512],
                KT[h][0:DK, u * P:(u + 1) * P],
                QT[h][qb][0:DK, n * 512:(n + 1) * 512],
                start=True, stop=True)
            nc.tensor.matmul(
                ps[:, 512:1024],
                KT[h][DK:P, u * P:(u + 1) * P],
                QT[h][qb][DK:P, n * 512:(n + 1) * 512],
                start=True, stop=True)
            pool, dtp = (p8_pool, fp8) if f8 else (pt_pool, bf16)
            pt = pool.tile([P, 1024], dtp, tag="p8" if f8 else "pt", name="pt")
            nc.scalar.activation(pt[:], ps[:], AF.Exp, scale=0.125)
            probs[s] = pt

        vq_of = {}

        def attnv_unit(t):
            bi, qb, j, par, h, n, u = _decode(t)
            f8 = h < FP8_HEADS
            key = (bi, n)
            if u == 0:
                vq_of[key] = pvq.tile([P, 512], f32, tag="vq", name="vq")
            vq = vq_of[key]
            pt = probs[t]
            lo = j * 192 + par * DK
            if f8:
                v8 = V8[u][:].rearrange("p (jj m) -> p jj m", jj=2)[:, :, lo:lo + P]
                nc.tensor.matmul(
                    vq[:], v8, pt[:].rearrange("p (jj q) -> p jj q", jj=2),
                    start=(u == 0), stop=(u == NU - 1), perf_mode=DR)
            else:
                for ci, cc in enumerate((2 * u, 2 * u + 1)):
                    nc.tensor.matmul(
                        vq[:], VB[cc][:, lo:lo + P],
                        pt[:, ci * 512:(ci + 1) * 512],
                        start=(cc == 0), stop=(cc == 15))
            if u == NU - 1:
                # normalize: vals/sums -> VT.  reciprocal_approx_fast (custom
                # DVE ucode) only works at partition base 0 on HW: stage the
                # sums there, recip, cross-copy up for odd heads.
                v_sl = slice(0, DK) if par == 0 else slice(DK, P)
                s_sl = slice(DK, P) if par == 0 else slice(0, DK)
                psl = slice(par * DK, (par + 1) * DK)
                su = rc_pool.tile([P, 512], f32, tag="su", name="su")
                rs = rc_pool.tile([P, 512], f32, tag="rs", name="rs")
                nc.vector.tensor_copy(su[0:DK, :], vq[s_sl, :])
                nc.vector.reciprocal_approx_fast(rs[0:DK, :], su[0:DK, :])
                if par:
                    nc.vector.tensor_copy(rs[DK:P, :], rs[0:DK, :])
                nc.vector.tensor_mul(
                    VT[j][psl, qb * 1024 + n * 512: qb * 1024 + (n + 1) * 512],
                    vq[v_sl, :], rs[psl, :])
                del vq_of[key], probs[t]

        # ================= phase 1: blocks 0-1 + V projection ===============
        # XT is only needed during phase 1 when Q/K contract x8; the bf16
        # Q/K path keeps reading it through the whole step loop.
        with ExitStack() as p1:
            xt_owner = p1 if QK_FP8 else ctx
            xt_pool = xt_owner.enter_context(tc.tile_pool(name="xt", bufs=1))
            wvp = p1.enter_context(tc.tile_pool(name="wvp", bufs=1))

            # same (sync) ring as the weights so these 4.5MB queue BEHIND the
            # critical x8/weight transfers instead of competing for HBM bw
            XTt, XTv = load_blocked(xt_pool, "xt", xT_d, 8, SEQ, bf16,
                                    nc.sync, ndma=2)
            XT = [XTv[:, k] for k in range(8)]
            _, WVv = load_blocked(wvp, "wv", wv_d, 8, 512, bf16, nc.sync)
            WV = [WVv[:, k] for k in range(8)]

            if use_b16:
                for cc in range(16):
                    nc.vector.memset(
                        VB[cc][:].rearrange("p (hp m) -> p hp m", m=192)[:, :, DK:2 * DK],
                        1.0)
            if use_f8:
                for u in range(8):
                    nc.vector.memset(
                        V8[u][:].rearrange("p (jj hp m) -> p jj hp m", jj=2, m=192)[:, :, :, DK:2 * DK],
                        1.0)

            vhalf_of = {}

            def vproj_q(cc, half):
                """V-projection quantum: token chunk cc, contraction half."""
                if half == 0:
                    vhalf_of[cc] = ppj.tile([P, 512], f32, tag="pj", name="pj")
                ps = vhalf_of[cc]
                for k in range(4 * half, 4 * half + 4):
                    nc.tensor.matmul(ps[:], XT[k][:, cc * P:(cc + 1) * P],
                                     WV[k], start=(k == 0), stop=(k == 7))
                if half == 1:
                    pv = ps[:].rearrange("p (hp par dd) -> p hp par dd",
                                         par=2, dd=DK)
                    if use_b16:
                        vb = VB[cc][:].rearrange("p (hp m) -> p hp m", m=192)
                        nc.vector.tensor_copy(vb[:, :, 0:DK], pv[:, :, 0, :])
                        nc.vector.tensor_copy(vb[:, :, 2 * DK:3 * DK], pv[:, :, 1, :])
                    if use_f8:
                        v8 = V8[cc // 2][:].rearrange(
                            "p (jj hp m) -> p jj hp m", jj=2, m=192)[:, cc % 2]
                        nc.vector.tensor_copy(v8[:, :, 0:DK], pv[:, :, 0, :])
                        nc.vector.tensor_copy(v8[:, :, 2 * DK:3 * DK], pv[:, :, 1, :])
                    del vhalf_of[cc]

            # ---- background quantum schedule ----
            bg = {}
            vdone = {}
            bg[0] = [lambda: kproj_q(0, 1, 0)]
            bg[1] = [lambda: kproj_q(0, 1, 1)]
            bg[2] = [lambda: qproj_q(0, 0, 1)]
            s = 12                       # V chunks 0..15, 2 quanta/step,
            for i in range(32):          # placed after XT's DMA has landed
                cc, hf = divmod(i, 2)
                bg.setdefault(s, []).append(lambda cc=cc, hf=hf: vproj_q(cc, hf))
                if hf:
                    vdone[cc] = s
                    s += 1
            pj1 = [(kproj_q, (1, 0, 0)), (kproj_q, (1, 0, 1)),
                   (kproj_q, (1, 1, 0)), (kproj_q, (1, 1, 1)),
                   (qproj_q, (1, 0, 0)), (qproj_q, (1, 0, 1))]
            for i, (fn, a) in enumerate(pj1):
                bg.setdefault(22 + 2 * i, []).append(lambda fn=fn, a=a: fn(*a))
            for base, j in ((34, 2), (58, 3)):
                for i, (kh, n) in enumerate(((0, 0), (0, 1), (1, 0), (1, 1))):
                    bg.setdefault(base + 3 * i, []).append(
                        lambda j=j, kh=kh, n=n: kproj_q(j, kh, n))
                for n in range(2):
                    bg.setdefault(base + 12 + 3 * n, []).append(
                        lambda j=j, n=n: qproj_q(j, 0, n))
            for i in range(4):                    # qb1 Q projections
                for n in range(2):
                    bg.setdefault(96 + 20 * i + 4 * n, []).append(
                        lambda j=i, n=n: qproj_q(j, 1, n))

            # attnv FIFO pacing: 1 unit/step; catch-up on alternating steps
            # once the V projection AND its K/Q neighbors are out of the way.
            av_sched = {}
            cur = 0
            for s in range(NSTEP + NSTEP // 2):
                cap = 1
                if s >= NSTEP or (s >= 48 and s % 2 == 0 and cur < s - 6):
                    cap = 2
                issued = 0
                while cur < NSTEP and issued < cap and cur <= s - LAG:
                    _, _, _, _, _, _, u_ = _decode(cur)
                    if cur < 2 * NU and s <= vdone.get(2 * u_ + 1, -1):
                        break
                    av_sched.setdefault(s, []).append(cur)
                    cur += 1
                    issued += 1

            # ---- prologue: wide K/Q bursts for block 0 on the score pool
            kproj_wide(0, 0)
            qproj_wide(0, 0)

            def step(s):
                scores_step(s)
                for fn in bg.get(s, ()):
                    fn()
                for t in av_sched.get(s, ()):
                    attnv_unit(t)

            for s in range(2 * 2 * NU):
                step(s)

        # ================= phase 2: blocks 2-15 + out projection ============
        wo_pool = ctx.enter_context(tc.tile_pool(name="wp", bufs=1))
        out_pool = ctx.enter_context(tc.tile_pool(name="op", bufs=3))
        mi2 = ctx.enter_context(tc.tile_pool(name="mi2", bufs=1))

        bo_s = mi2.tile([P, DM], f32, tag="bo", name="bo")
        nc.gpsimd.dma_start(bo_s[:], bo_d[:])
        _, WOv = load_blocked(wo_pool, "wo", wo_d, 4, DM, bf16, nc.gpsimd)

        def oproj(m):
            """Out-projection for token chunk m: j-outer/n-inner so each VT
            stationary is loaded once and shared by the two n-halves."""
            po = [ppj.tile([P, 512], f32, tag="pj", name="pj") for _ in range(2)]
            for j in range(4):
                for n in range(2):
                    nc.tensor.matmul(
                        po[n][:], VT[j][:, m * P:(m + 1) * P],
                        WOv[:, j, n * 512:(n + 1) * 512],
                        start=(j == 0), stop=(j == 3))
            for n in range(2):
                ot = out_pool.tile([P, 512], f32, tag="ot", name="ot")
                nc.vector.tensor_add(ot[:], po[n][:], bo_s[:, n * 512:(n + 1) * 512])
                nc.sync.dma_start(
                    out_d[m * P:(m + 1) * P, n * 512:(n + 1) * 512], ot[:])

        # qb0 out-projection after attnv unit 127 (step ~130)
        for m in range(8):
            bg.setdefault(138 + 4 * m, []).append(lambda m=m: oproj(m))
        # tokens 1024:1536 (m 8..11) only need the qb1 n=0 attnv finishes,
        # the last of which is unit 247 — overlap those with the final steps
        s247 = next(s for s in sorted(av_sched) if 247 in av_sched[s])
        for i in range(4):
            bg.setdefault(min(s247 + 1 + i, NSTEP - 1), []).append(
                lambda m=8 + i: oproj(m))

        for s in range(2 * 2 * NU, NSTEP):
            step(s)
        for s in range(NSTEP, NSTEP + NSTEP // 2):
            for t in av_sched.get(s, ()):
                attnv_unit(t)
        for m in range(12, 16):
            oproj(m)

        if DBG:
            for h in range(H8):
                nc.sync.dma_start(dkt_d[:, h * 1024:(h + 1) * 1024], KT[h][:])
                for qb in range(NQB):
                    i = h * NQB + qb
                    nc.sync.dma_start(dqt_d[:, i * 1024:(i + 1) * 1024],
                                      QT[h][qb][:])
            if use_b16:
                for cc in range(16):
                    nc.sync.dma_start(dvb_d[:, cc * 768:(cc + 1) * 768], VB[cc][:])
            for j in range(NPAIR):
                nc.sync.dma_start(dvt_d[:, j * SEQ:(j + 1) * SEQ], VT[j][:])

    nc.compile()
    return nc


def _get_nc():
    if "nc" not in _CACHE:
        _CACHE["nc"] = _build()
    return _CACHE["nc"]


def _prep_weights(W_qkv, b_qkv, W_o, b_o, hh):
    W3 = np.asarray(W_qkv, np.float32).reshape(H, 3 * DK, DM)
    hs = slice(hh * H8, (hh + 1) * H8)
    Wq = W3[hs, 0:DK, :].reshape(512, DM)
    Wk = W3[hs, DK:2 * DK, :].reshape(512, DM)
    Wv = W3[hs, 2 * DK:3 * DK, :].reshape(512, DM)
    b3 = np.asarray(b_qkv, np.float32).reshape(H, 3 * DK)
    bq = b3[hs, 0:DK].reshape(512)
    bk = b3[hs, DK:2 * DK].reshape(512)
    bv = b3[hs, 2 * DK:3 * DK].reshape(512)
    Wo_c = np.asarray(W_o, np.float32)[:, hh * 512:(hh + 1) * 512]
    bt = Wo_c @ bv + (np.asarray(b_o, np.float32) if hh == 0 else 0.0)

    def bake(a):  # [nblk*128, cols] -> [128, nblk*cols] SBUF layout
        nb = a.shape[0] // P
        return np.ascontiguousarray(
            a.reshape(nb, P, a.shape[1]).transpose(1, 0, 2).reshape(P, -1))

    def dr_pack(WT):  # [1024, m] -> [512, 2*m] DoubleRow layout
        m = WT.shape[1]
        return np.ascontiguousarray(
            WT.reshape(4, 2, P, m).transpose(0, 2, 1, 3).reshape(4 * P, 2 * m))

    wm = {
        "wvT": bake(Wv.T).astype(_BF16),
        "woT": bake(Wo_c.T).astype(_BF16),
        "bq4": np.ascontiguousarray(bq.reshape(4, P).T, dtype=np.float32),
        "bk4": np.ascontiguousarray(bk.reshape(4, P).T, dtype=np.float32),
        "bob": np.ascontiguousarray(np.tile(bt[None, :], (P, 1)), dtype=np.float32),
    }
    if QK_FP8:
        wm["wq8"] = bake(dr_pack(Wq.T)).astype(_FP8)
        wm["wk8"] = bake(dr_pack(Wk.T)).astype(_FP8)
    else:
        wm["wqT"] = bake(Wq.T).astype(_BF16)
        wm["wkT"] = bake(Wk.T).astype(_BF16)
    return wm


def make_in_maps(x, W_qkv, b_qkv, W_o, b_o):
    x = np.asarray(x, np.float32)
    wms = [_prep_weights(W_qkv, b_qkv, W_o, b_o, hh) for hh in range(2)]
    in_maps = []
    xbk, x8bk = [], []
    for b in range(4):
        xT = np.ascontiguousarray(x[b].T)
        xbk.append(np.ascontiguousarray(
            xT.reshape(8, P, SEQ).transpose(1, 0, 2).reshape(P, 8 * SEQ)
        ).astype(_BF16))
        if QK_FP8:
            dr = xT.reshape(4, 2, P, SEQ).transpose(0, 2, 1, 3).reshape(4 * P, 2 * SEQ)
            x8bk.append(np.ascontiguousarray(
                dr.reshape(4, P, 2, 2, 1024).transpose(1, 3, 0, 2, 4).reshape(P, 8 * SEQ)
            ).astype(_FP8))
    for c in range(NCORES):
        b, hh = divmod(c, 2)
        m = {"xT": xbk[b], **wms[hh]}
        if QK_FP8:
            m["xT8"] = x8bk[b]
        in_maps.append(m)
    return in_maps


def assemble(results):
    out = np.empty((4, SEQ, DM), np.float32)
    for b in range(4):
        out[b] = results[2 * b]["out"]
        out[b] += results[2 * b + 1]["out"]
    return out


def kernel(x, mask, W_qkv, b_qkv, W_o, b_o):
    from concourse.bass_utils import run_bass_kernel_spmd

    nc = _get_nc()
    in_maps = make_in_maps(x, W_qkv, b_qkv, W_o, b_o)
    res = run_bass_kernel_spmd(nc, in_maps, list(range(NCORES)))
    return assemble(res.results)



# revision 4
# speedup vs baseline: 1.0217x; 1.0217x over previous
"""Multi-head attention (B=4, S=2048, D=1024, H=16) on 8 NeuronCores — v3.2.

Sharding: core c = (batch b = c//2, head-half hh = c%2).  Each core computes
Q/K/V for its 8 heads over the FULL sequence, then a PARTIAL output
projection; the two cores of a batch pair are summed on the host (the
tensor-parallel all-reduce).

Pipeline: 256 steps, one per (head, qblock, query-half n, chunk-pair u).
Each step's two score matmuls — even key chunk 2u (KT partitions 0:64, PE
row group 0) and odd chunk 2u+1 (partitions 64:128, row group 1) — write
one [128,1024] PSUM tile freed by ONE exp, so they become ready together,
sit adjacent in the PE queue, and run concurrently in disjoint 64-row PE
groups (row tiling ~2x).  QT is duplicated into both partition halves to
match the stationary's row group.  attn@V units are FIFO-paced (elastic
lag) and issued in pairs so same-stationary-class MMs batch (fewer exposed
LDWEIGHTS).  All projections run as ~4-8 MM quanta spread across steps so
the exp stream (~1us per [128,1024], the pace-setter) never stalls.

Inputs arrive as ONE large strided DMA per tensor (row-blocks of the DRAM
tensor gathered into free-dim slots) — 10 issues instead of 50, avoiding
the shared-semaphore-pool serialization that cost ~35us of prologue.

QK_FP8: Q/K projections contract x in fp8e4 DoubleRow (256 dims/MM).
FP8_HEADS: heads below this index run attn@V in fp8 DoubleRow (one MM per
chunk-pair, 256 keys) with exp writing fp8 probs directly.  Error budget
(measured, rel): bf16-only 0.0023; +QK fp8 0.0164; +attnv fp8 0.0187;
both 0.0248 (fails 2e-2) — enable at most one.

Softmax: row sums via a ones-block in the attn@V stationary (M<=128 adds
no PE cycles); normalization = DVE reciprocal_approx_fast (must run at
partition base 0 — custom-ucode quirk) + tensor_mul.  V-bias is folded
into the out bias host-side (attention rows sum to 1).
"""

import numpy as np
import ml_dtypes
from contextlib import ExitStack

P = 128
DM = 1024
SEQ = 2048
H = 16
H8 = 8          # heads per core
NPAIR = 4      # head pairs per core
DK = 64
NCORES = 8
NQB = 2         # query blocks of 1024
NU = 8          # chunk-pair steps per (head, qblock, n-half)
NSTEP = H8 * NQB * 2 * NU   # 256
LAG = 3         # min attnv trail (steps)

FP8_HEADS = 0   # per-core heads (from index 0) using fp8 DoubleRow attn@V
QK_FP8 = True   # Q/K projections contract x in fp8 DoubleRow
DBG = False     # add DRAM taps of intermediate tiles

_BF16 = ml_dtypes.bfloat16
_FP8 = ml_dtypes.float8_e4m3

_CACHE = {}


def _decode(t):
    """step/unit index -> (bi, qb, j, par, h, n, u)"""
    bi, w = divmod(t, 2 * NU)
    qb, jj = divmod(bi, H8)
    j, par = divmod(jj, 2)
    n, u = divmod(w, NU)
    return bi, qb, j, par, 2 * j + par, n, u


def _build():
    from concourse import bacc
    import concourse.mybir as mybir
    from concourse.tile import TileContext

    dt = mybir.dt
    f32 = dt.float32
    bf16 = dt.bfloat16
    fp8 = dt.float8e4
    AF = mybir.ActivationFunctionType
    DR = mybir.MatmulPerfMode.DoubleRow

    nc = bacc.Bacc("TRN2", target_bir_lowering=False, debug=False)

    # all inputs arrive pre-baked in their SBUF layout ([128, nblk*cols],
    # row-blocks in the free dim) so each is ONE contiguous full-bw DMA
    xT_d = nc.dram_tensor("xT", [P, 8 * SEQ], bf16, kind="ExternalInput")
    wv_d = nc.dram_tensor("wvT", [P, 8 * 512], bf16, kind="ExternalInput")
    wo_d = nc.dram_tensor("woT", [P, 4 * DM], bf16, kind="ExternalInput")
    bq_d = nc.dram_tensor("bq4", [P, NPAIR], f32, kind="ExternalInput")
    bk_d = nc.dram_tensor("bk4", [P, NPAIR], f32, kind="ExternalInput")
    bo_d = nc.dram_tensor("bob", [P, DM], f32, kind="ExternalInput")
    if QK_FP8:
        x8_d = nc.dram_tensor("xT8", [P, 4 * 2 * SEQ], fp8, kind="ExternalInput")
        wq_d = nc.dram_tensor("wq8", [P, 4 * 2 * 512], fp8, kind="ExternalInput")
        wk_d = nc.dram_tensor("wk8", [P, 4 * 2 * 512], fp8, kind="ExternalInput")
    else:
        wq_d = nc.dram_tensor("wqT", [P, 8 * 512], bf16, kind="ExternalInput")
        wk_d = nc.dram_tensor("wkT", [P, 8 * 512], bf16, kind="ExternalInput")
    out_d = nc.dram_tensor("out", [SEQ, DM], bf16, kind="ExternalOutput")
    if DBG:
        dkt_d = nc.dram_tensor("dkt", [P, 1024 * H8], bf16, kind="ExternalOutput")
        dqt_d = nc.dram_tensor("dqt", [P, 1024 * H8 * NQB], bf16, kind="ExternalOutput")
        dvb_d = nc.dram_tensor("dvb", [P, 768 * 16], bf16, kind="ExternalOutput")
        dvt_d = nc.dram_tensor("dvt", [P, SEQ * NPAIR], bf16, kind="ExternalOutput")

    use_f8 = FP8_HEADS > 0
    use_b16 = FP8_HEADS < H8

    with TileContext(nc) as tc, ExitStack() as ctx:
        # ---- permanent pools ----
        kt_pool = ctx.enter_context(tc.tile_pool(name="kt", bufs=H8))
        qt_pool = ctx.enter_context(tc.tile_pool(name="qt", bufs=H8 * NQB))
        vt_pool = ctx.enter_context(tc.tile_pool(name="vt", bufs=NPAIR))
        vb_pool = ctx.enter_context(tc.tile_pool(name="vb", bufs=16 if use_b16 else 1))
        v8_pool = ctx.enter_context(tc.tile_pool(name="v8", bufs=8 if use_f8 else 1))
        pt_pool = ctx.enter_context(tc.tile_pool(name="pt", bufs=20 if use_b16 else 1))
        p8_pool = ctx.enter_context(tc.tile_pool(name="p8", bufs=20 if use_f8 else 1))
        rc_pool = ctx.enter_context(tc.tile_pool(name="rc", bufs=2))
        misc = ctx.enter_context(tc.tile_pool(name="mi", bufs=1))
        wqk_pool = ctx.enter_context(tc.tile_pool(name="w8", bufs=1))
        if QK_FP8:
            x8_pool = ctx.enter_context(tc.tile_pool(name="x8", bufs=1))
        # PSUM (8 banks): scores 2x[128,1024] (4) + proj 2x[128,512] (2)
        #                 + attnv accum 2x[128,512] (2)
        psc = ctx.enter_context(tc.tile_pool(name="psc", bufs=2, space="PSUM"))
        ppj = ctx.enter_context(tc.tile_pool(name="ppj", bufs=2, space="PSUM"))
        pvq = ctx.enter_context(tc.tile_pool(name="pvq", bufs=2, space="PSUM"))

        KT = [kt_pool.tile([P, 1024], bf16, tag="kt", name="kt") for _ in range(H8)]
        QT = [[qt_pool.tile([P, 1024], bf16, tag="qt", name="qt")
               for _ in range(NQB)] for _ in range(H8)]
        VT = [vt_pool.tile([P, SEQ], bf16, tag="vt", name="vt") for _ in range(NPAIR)]
        VB = ([vb_pool.tile([P, NPAIR * 192], bf16, tag="vb", name="vb")
               for _ in range(16)] if use_b16 else None)
        V8 = ([v8_pool.tile([P, 2 * NPAIR * 192], fp8, tag="v8", name="v8")
               for _ in range(8)] if use_f8 else None)

        # ---- consolidated input DMAs (one contiguous transfer per tensor).
        # Scalar queue carries ONLY activations; weights/x8 on sync.
        def load_blocked(pool, tag, dram, nblk, cols, dtp, eng, ndma=1):
            t = pool.tile([P, nblk * cols], dtp, tag=tag, name=tag)
            n = nblk * cols
            for i in range(ndma):
                sl = slice(i * n // ndma, (i + 1) * n // ndma)
                eng.dma_start(t[:, sl], dram[:, sl])
            return t, t[:].rearrange("p (k c) -> p k c", k=nblk)

        if QK_FP8:
            # x8 free-dim order (thalf, k2, jj, 1024): the prologue's K/Q
            # bursts (keys/queries 0:1024) unblock after the first 1MB dma
            _, WKv = load_blocked(wqk_pool, "wk8", wk_d, 4, 1024, fp8, nc.sync)
            x8t = x8_pool.tile([P, 4 * 2 * SEQ], fp8, tag="x8", name="x8")
            nc.sync.dma_start(x8t[:, 0:8192], x8_d[:, 0:8192])
            _, WQv = load_blocked(wqk_pool, "wq8", wq_d, 4, 1024, fp8, nc.sync)
            X8v = x8t[:].rearrange("p (hf k2 jj t) -> p hf k2 jj t", hf=2, k2=4, jj=2)
        else:
            _, WKv = load_blocked(wqk_pool, "wkT", wk_d, 8, 512, bf16, nc.sync)
            _, WQv = load_blocked(wqk_pool, "wqT", wq_d, 8, 512, bf16, nc.sync)
        bq_s = misc.tile([P, NPAIR], f32, tag="bq", name="bq")
        nc.sync.dma_start(bq_s[:], bq_d[:])
        bk_s = misc.tile([P, NPAIR], f32, tag="bk", name="bk")
        nc.sync.dma_start(bk_s[:], bk_d[:])
        if QK_FP8:
            nc.sync.dma_start(x8t[:, 8192:16384], x8_d[:, 8192:16384])

        if QK_FP8:
            def qk_mms(WV_, j, ps, psl, cols):
                hf, tt = divmod(cols.start, 1024)
                for k2 in range(4):
                    w = WV_[:, k2].rearrange("p (jj m) -> p jj m", jj=2)[:, :, j * P:(j + 1) * P]
                    x = X8v[:, hf, k2, :, tt:tt + 512]
                    nc.tensor.matmul(ps[:, psl], w, x, start=(k2 == 0),
                                     stop=(k2 == 3), perf_mode=DR)
        else:
            def qk_mms(WV_, j, ps, psl, cols):
                for k in range(8):
                    nc.tensor.matmul(
                        ps[:, psl], WV_[:, k, j * P:(j + 1) * P],
                        XT[k][:, cols], start=(k == 0), stop=(k == 7))

        def k_dve(ps, j, kh, n):
            """parity-split DVE writes of a K-proj [128, 512*w] psum region
            covering chunks from (kh*8 + n*4)."""
            pw = ps.rearrange("p (u par i) -> p u par i", par=2, i=P)
            c0 = (kh * 4 + n * 2) * P
            w = ps.shape[1] // 256
            for hp in range(2):
                h = 2 * j + hp
                r0 = hp * DK
                for par in range(2):
                    dst = KT[h][par * DK:(par + 1) * DK, c0:c0 + w * P]
                    nc.vector.tensor_scalar_add(
                        dst.rearrange("p (u i) -> p u i", i=P),
                        pw[r0:r0 + DK, :, par, :], bk_s[r0:r0 + DK, j:j + 1])

        def q_dve(ps, j, qb, n, wd):
            for hp in range(2):
                h = 2 * j + hp
                r0 = hp * DK
                for half in range(2):
                    nc.vector.tensor_scalar_add(
                        QT[h][qb][half * DK:(half + 1) * DK,
                                  n * 512:n * 512 + wd],
                        ps[r0:r0 + DK, :], bq_s[r0:r0 + DK, j:j + 1])

        def kproj_q(j, kh, n):
            ps = ppj.tile([P, 512], f32, tag="pj", name="pj")
            qk_mms(WKv, j, ps, slice(0, 512),
                   slice(kh * 1024 + n * 512, kh * 1024 + (n + 1) * 512))
            k_dve(ps[:], j, kh, n)

        def qproj_q(j, qb, n):
            ps = ppj.tile([P, 512], f32, tag="pj", name="pj")
            qk_mms(WQv, j, ps, slice(0, 512),
                   slice(qb * 1024 + n * 512, qb * 1024 + (n + 1) * 512))
            q_dve(ps[:], j, qb, n, 512)

        def kproj_wide(j, kh):
            """prologue K-proj half on the (idle) score psum pool."""
            ps = psc.tile([P, 1024], f32, tag="sc", name="sc")
            for n in range(2):
                qk_mms(WKv, j, ps, slice(n * 512, (n + 1) * 512),
                       slice(kh * 1024 + n * 512, kh * 1024 + (n + 1) * 512))
            k_dve(ps[:], j, kh, 0)

        def qproj_wide(j, qb):
            ps = psc.tile([P, 1024], f32, tag="sc", name="sc")
            for n in range(2):
                qk_mms(WQv, j, ps, slice(n * 512, (n + 1) * 512),
                       slice(qb * 1024 + n * 512, qb * 1024 + (n + 1) * 512))
            q_dve(ps[:], j, qb, 0, 1024)

        probs = {}

        def scores_step(s):
            bi, qb, j, par, h, n, u = _decode(s)
            f8 = h < FP8_HEADS
            ps = psc.tile([P, 1024], f32, tag="sc", name="sc")
            nc.tensor.matmul(
                ps[:, 0:512],
                KT[h][0:DK, u * P:(u + 1) * P],
                QT[h][qb][0:DK, n * 512:(n + 1) * 512],
                start=True, stop=True)
            nc.tensor.matmul(
                ps[:, 512:1024],
                KT[h][DK:P, u * P:(u + 1) * P],
                QT[h][qb][DK:P, n * 512:(n + 1) * 512],
                start=True, stop=True)
            pool, dtp = (p8_pool, fp8) if f8 else (pt_pool, bf16)
            pt = pool.tile([P, 1024], dtp, tag="p8" if f8 else "pt", name="pt")
            nc.scalar.activation(pt[:], ps[:], AF.Exp, scale=0.125)
            probs[s] = pt

        vq_of = {}

        def attnv_unit(t):
            bi, qb, j, par, h, n, u = _decode(t)
            f8 = h < FP8_HEADS
            key = (bi, n)
            if u == 0:
                vq_of[key] = pvq.tile([P, 512], f32, tag="vq", name="vq")
            vq = vq_of[key]
            pt = probs[t]
            lo = j * 192 + par * DK
            if f8:
                v8 = V8[u][:].rearrange("p (jj m) -> p jj m", jj=2)[:, :, lo:lo + P]
                nc.tensor.matmul(
                    vq[:], v8, pt[:].rearrange("p (jj q) -> p jj q", jj=2),
                    start=(u == 0), stop=(u == NU - 1), perf_mode=DR)
            else:
                for ci, cc in enumerate((2 * u, 2 * u + 1)):
                    nc.tensor.matmul(
                        vq[:], VB[cc][:, lo:lo + P],
                        pt[:, ci * 512:(ci + 1) * 512],
                        start=(cc == 0), stop=(cc == 15))
            if u == NU - 1:
                # normalize: vals/sums -> VT.  reciprocal_approx_fast (custom
                # DVE ucode) only works at partition base 0 on HW: stage the
                # sums there, recip, cross-copy up for odd heads.
                v_sl = slice(0, DK) if par == 0 else slice(DK, P)
                s_sl = slice(DK, P) if par == 0 else slice(0, DK)
                psl = slice(par * DK, (par + 1) * DK)
                su = rc_pool.tile([P, 512], f32, tag="su", name="su")
                rs = rc_pool.tile([P, 512], f32, tag="rs", name="rs")
                nc.vector.tensor_copy(su[0:DK, :], vq[s_sl, :])
                nc.vector.reciprocal_approx_fast(rs[0:DK, :], su[0:DK, :])
                if par:
                    nc.vector.tensor_copy(rs[DK:P, :], rs[0:DK, :])
                nc.vector.tensor_mul(
                    VT[j][psl, qb * 1024 + n * 512: qb * 1024 + (n + 1) * 512],
                    vq[v_sl, :], rs[psl, :])
                del vq_of[key], probs[t]

        # ================= phase 1: blocks 0-1 + V projection ===============
        # XT is only needed during phase 1 when Q/K contract x8; the bf16
        # Q/K path keeps reading it through the whole step loop.
        with ExitStack() as p1:
            xt_owner = p1 if QK_FP8 else ctx
            xt_pool = xt_owner.enter_context(tc.tile_pool(name="xt", bufs=1))
            wvp = p1.enter_context(tc.tile_pool(name="wvp", bufs=1))

            # same (sync) ring as the weights so these 4.5MB queue BEHIND the
            # critical x8/weight transfers instead of competing for HBM bw
            XTt, XTv = load_blocked(xt_pool, "xt", xT_d, 8, SEQ, bf16,
                                    nc.sync, ndma=2)
            XT = [XTv[:, k] for k in range(8)]
            _, WVv = load_blocked(wvp, "wv", wv_d, 8, 512, bf16, nc.sync)
            WV = [WVv[:, k] for k in range(8)]

            if use_b16:
                for cc in range(16):
                    nc.vector.memset(
                        VB[cc][:].rearrange("p (hp m) -> p hp m", m=192)[:, :, DK:2 * DK],
                        1.0)
            if use_f8:
                for u in range(8):
                    nc.vector.memset(
                        V8[u][:].rearrange("p (jj hp m) -> p jj hp m", jj=2, m=192)[:, :, :, DK:2 * DK],
                        1.0)

            vhalf_of = {}

            def vproj_q(cc, half):
                """V-projection quantum: token chunk cc, contraction half."""
                if half == 0:
                    vhalf_of[cc] = ppj.tile([P, 512], f32, tag="pj", name="pj")
                ps = vhalf_of[cc]
                for k in range(4 * half, 4 * half + 4):
                    nc.tensor.matmul(ps[:], XT[k][:, cc * P:(cc + 1) * P],
                                     WV[k], start=(k == 0), stop=(k == 7))
                if half == 1:
                    pv = ps[:].rearrange("p (hp par dd) -> p hp par dd",
                                         par=2, dd=DK)
                    if use_b16:
                        vb = VB[cc][:].rearrange("p (hp m) -> p hp m", m=192)
                        nc.vector.tensor_copy(vb[:, :, 0:DK], pv[:, :, 0, :])
                        nc.vector.tensor_copy(vb[:, :, 2 * DK:3 * DK], pv[:, :, 1, :])
                    if use_f8:
                        v8 = V8[cc // 2][:].rearrange(
                            "p (jj hp m) -> p jj hp m", jj=2, m=192)[:, cc % 2]
                        nc.vector.tensor_copy(v8[:, :, 0:DK], pv[:, :, 0, :])
                        nc.vector.tensor_copy(v8[:, :, 2 * DK:3 * DK], pv[:, :, 1, :])
                    del vhalf_of[cc]

            # ---- background quantum schedule ----
            bg = {}
            vdone = {}
            bg[0] = [lambda: kproj_q(0, 1, 0)]
            bg[1] = [lambda: kproj_q(0, 1, 1)]
            bg[2] = [lambda: qproj_q(0, 0, 1)]
            s = 12                       # V chunks 0..15, 2 quanta/step,
            for i in range(32):          # placed after XT's DMA has landed
                cc, hf = divmod(i, 2)
                bg.setdefault(s, []).append(lambda cc=cc, hf=hf: vproj_q(cc, hf))
                if hf:
                    vdone[cc] = s
                    s += 1
            pj1 = [(kproj_q, (1, 0, 0)), (kproj_q, (1, 0, 1)),
                   (kproj_q, (1, 1, 0)), (kproj_q, (1, 1, 1)),
                   (qproj_q, (1, 0, 0)), (qproj_q, (1, 0, 1))]
            for i, (fn, a) in enumerate(pj1):
                bg.setdefault(22 + 2 * i, []).append(lambda fn=fn, a=a: fn(*a))
            for base, j in ((34, 2), (58, 3)):
                for i, (kh, n) in enumerate(((0, 0), (0, 1), (1, 0), (1, 1))):
                    bg.setdefault(base + 3 * i, []).append(
                        lambda j=j, kh=kh, n=n: kproj_q(j, kh, n))
                for n in range(2):
                    bg.setdefault(base + 12 + 3 * n, []).append(
                        lambda j=j, n=n: qproj_q(j, 0, n))
            for i in range(4):                    # qb1 Q projections
                for n in range(2):
                    bg.setdefault(96 + 20 * i + 4 * n, []).append(
                        lambda j=i, n=n: qproj_q(j, 1, n))

            # attnv FIFO pacing: 1 unit/step; catch-up on alternating steps
            # once the V projection AND its K/Q neighbors are out of the way.
            av_sched = {}
            cur = 0
            for s in range(NSTEP + NSTEP // 2):
                cap = 1
                if s >= NSTEP or (s >= 48 and s % 2 == 0 and cur < s - 6):
                    cap = 2
                issued = 0
                while cur < NSTEP and issued < cap and cur <= s - LAG:
                    _, _, _, _, _, _, u_ = _decode(cur)
                    if cur < 2 * NU and s <= vdone.get(2 * u_ + 1, -1):
                        break
                    av_sched.setdefault(s, []).append(cur)
                    cur += 1
                    issued += 1

            # ---- prologue: wide K/Q bursts for block 0 on the score pool
            kproj_wide(0, 0)
            qproj_wide(0, 0)

            def step(s):
                scores_step(s)
                for fn in bg.get(s, ()):
                    fn()
                for t in av_sched.get(s, ()):
                    attnv_unit(t)

            for s in range(2 * 2 * NU):
                step(s)

        # ================= phase 2: blocks 2-15 + out projection ============
        wo_pool = ctx.enter_context(tc.tile_pool(name="wp", bufs=1))
        out_pool = ctx.enter_context(tc.tile_pool(name="op", bufs=3))
        mi2 = ctx.enter_context(tc.tile_pool(name="mi2", bufs=1))

        bo_s = mi2.tile([P, DM], f32, tag="bo", name="bo")
        nc.gpsimd.dma_start(bo_s[:], bo_d[:])
        _, WOv = load_blocked(wo_pool, "wo", wo_d, 4, DM, bf16, nc.gpsimd)

        def oproj(m):
            """Out-projection for token chunk m: j-outer/n-inner so each VT
            stationary is loaded once and shared by the two n-halves."""
            po = [ppj.tile([P, 512], f32, tag="pj", name="pj") for _ in range(2)]
            for j in range(4):
                for n in range(2):
                    nc.tensor.matmul(
                        po[n][:], VT[j][:, m * P:(m + 1) * P],
                        WOv[:, j, n * 512:(n + 1) * 512],
                        start=(j == 0), stop=(j == 3))
            for n in range(2):
                ot = out_pool.tile([P, 512], bf16, tag="ot", name="ot")
                nc.vector.tensor_add(ot[:], po[n][:], bo_s[:, n * 512:(n + 1) * 512])
                nc.sync.dma_start(
                    out_d[m * P:(m + 1) * P, n * 512:(n + 1) * 512], ot[:])

        # qb0 out-projection after attnv unit 127 (step ~130)
        for m in range(8):
            bg.setdefault(138 + 4 * m, []).append(lambda m=m: oproj(m))
        # tokens 1024:1536 (m 8..11) only need the qb1 n=0 attnv finishes,
        # the last of which is unit 247 — overlap those with the final steps
        s247 = next(s for s in sorted(av_sched) if 247 in av_sched[s])
        for i in range(4):
            bg.setdefault(min(s247 + 1 + i, NSTEP - 1), []).append(
                lambda m=8 + i: oproj(m))

        for s in range(2 * 2 * NU, NSTEP):
            step(s)
        for s in range(NSTEP, NSTEP + NSTEP // 2):
            for t in av_sched.get(s, ()):
                attnv_unit(t)
        for m in range(12, 16):
            oproj(m)

        if DBG:
            for h in range(H8):
                nc.sync.dma_start(dkt_d[:, h * 1024:(h + 1) * 1024], KT[h][:])
                for qb in range(NQB):
                    i = h * NQB + qb
                    nc.sync.dma_start(dqt_d[:, i * 1024:(i + 1) * 1024],
                                      QT[h][qb][:])
            if use_b16:
                for cc in range(16):
                    nc.sync.dma_start(dvb_d[:, cc * 768:(cc + 1) * 768], VB[cc][:])
            for j in range(NPAIR):
                nc.sync.dma_start(dvt_d[:, j * SEQ:(j + 1) * SEQ], VT[j][:])

    nc.compile()
    return nc


def _get_nc():
    if "nc" not in _CACHE:
        _CACHE["nc"] = _build()
    return _CACHE["nc"]


def _prep_weights(W_qkv, b_qkv, W_o, b_o, hh):
    W3 = np.asarray(W_qkv, np.float32).reshape(H, 3 * DK, DM)
    hs = slice(hh * H8, (hh + 1) * H8)
    Wq = W3[hs, 0:DK, :].reshape(512, DM)
    Wk = W3[hs, DK:2 * DK, :].reshape(512, DM)
    Wv = W3[hs, 2 * DK:3 * DK, :].reshape(512, DM)
    b3 = np.asarray(b_qkv, np.float32).reshape(H, 3 * DK)
    bq = b3[hs, 0:DK].reshape(512)
    bk = b3[hs, DK:2 * DK].reshape(512)
    bv = b3[hs, 2 * DK:3 * DK].reshape(512)
    Wo_c = np.asarray(W_o, np.float32)[:, hh * 512:(hh + 1) * 512]
    bt = Wo_c @ bv + (np.asarray(b_o, np.float32) if hh == 0 else 0.0)

    def bake(a):  # [nblk*128, cols] -> [128, nblk*cols] SBUF layout
        nb = a.shape[0] // P
        return np.ascontiguousarray(
            a.reshape(nb, P, a.shape[1]).transpose(1, 0, 2).reshape(P, -1))

    def dr_pack(WT):  # [1024, m] -> [512, 2*m] DoubleRow layout
        m = WT.shape[1]
        return np.ascontiguousarray(
            WT.reshape(4, 2, P, m).transpose(0, 2, 1, 3).reshape(4 * P, 2 * m))

    wm = {
        "wvT": bake(Wv.T).astype(_BF16),
        "woT": bake(Wo_c.T).astype(_BF16),
        "bq4": np.ascontiguousarray(bq.reshape(4, P).T, dtype=np.float32),
        "bk4": np.ascontiguousarray(bk.reshape(4, P).T, dtype=np.float32),
        "bob": np.ascontiguousarray(np.tile(bt[None, :], (P, 1)), dtype=np.float32),
    }
    if QK_FP8:
        wm["wq8"] = bake(dr_pack(Wq.T)).astype(_FP8)
        wm["wk8"] = bake(dr_pack(Wk.T)).astype(_FP8)
    else:
        wm["wqT"] = bake(Wq.T).astype(_BF16)
        wm["wkT"] = bake(Wk.T).astype(_BF16)
    return wm


def make_in_maps(x, W_qkv, b_qkv, W_o, b_o):
    x = np.asarray(x, np.float32)
    wms = [_prep_weights(W_qkv, b_qkv, W_o, b_o, hh) for hh in range(2)]
    in_maps = []
    xbk, x8bk = [], []
    for b in range(4):
        xT = np.ascontiguousarray(x[b].T)
        xbk.append(np.ascontiguousarray(
            xT.reshape(8, P, SEQ).transpose(1, 0, 2).reshape(P, 8 * SEQ)
        ).astype(_BF16))
        if QK_FP8:
            dr = xT.reshape(4, 2, P, SEQ).transpose(0, 2, 1, 3).reshape(4 * P, 2 * SEQ)
            x8bk.append(np.ascontiguousarray(
                dr.reshape(4, P, 2, 2, 1024).transpose(1, 3, 0, 2, 4).reshape(P, 8 * SEQ)
            ).astype(_FP8))
    for c in range(NCORES):
        b, hh = divmod(c, 2)
        m = {"xT": xbk[b], **wms[hh]}
        if QK_FP8:
            m["xT8"] = x8bk[b]
        in_maps.append(m)
    return in_maps


def assemble(results):
    out = np.empty((4, SEQ, DM), np.float32)
    for b in range(4):
        out[b] = np.asarray(results[2 * b]["out"], np.float32)
        out[b] += np.asarray(results[2 * b + 1]["out"], np.float32)
    return out


def kernel(x, mask, W_qkv, b_qkv, W_o, b_o):
    from concourse.bass_utils import run_bass_kernel_spmd

    nc = _get_nc()
    in_maps = make_in_maps(x, W_qkv, b_qkv, W_o, b_o)
    res = run_bass_kernel_spmd(nc, in_maps, list(range(NCORES)))
    return assemble(res.results)

